# revision 22
# baseline (speedup 1.0000x reference)
"""GRU observation-cell kernel for Trainium2 (8 NeuronCores).

Reference computation:
    x = X_obs.reshape(M, 128); hs = h[i_obs]
    h_new = GRUCell(x, hs)  (torch gate order r,z,n)
    out = h.at[i_obs].set(h_new)

Device strategy (data parallel over observed rows, per sharding hint):
  - M=20000 observed rows sharded 2500/core across 8 cores.
  - Gates-on-partitions layout: host supplies x^T [128,2500] and hs^T
    [256,2500] per core, so every matmul operand is already contract-
    dim-major and no on-chip transposes are needed.
  - Matmuls run as float32r (fp32 bits, full PE rate at >=256 moving dim),
    accumulating fp32 in PSUM. For the r/z gates both x@W_ih^T and
    hs@W_hh^T accumulate into the same PSUM bank, so the gi+gh add is free.
  - n gate: t1 = r*(h_n + b_hh_n) is one fused scalar_tensor_tensor on
    DVE; the + i_n add rides on the PE as an identity-matmul accumulate
    into the i_n PSUM bank, and tanh(+b_ih_n bias) drains that bank.
  - Emission order r-gates -> n-gates -> z-gates shortens the critical
    path into the DVE chain.
  - The gather h[i_obs] / scatter out[i_obs] is part of host-side
    shard/unshard (i_obs indexes rows; untouched rows pass through).
"""

import numpy as np

N, H, IN2, M, NCORES = 100000, 256, 128, 20000, 8
MC = M // NCORES        # 2500 observed rows per core
RT = 500                # rows per tile (<=512 fp32 PSUM bank limit)
NRT = MC // RT          # 5 row tiles per core
G3 = 3 * H              # 768 stacked gates (r,z,n)

_compiled = {}


def _build_nc():
    from contextlib import ExitStack

    from concourse import bacc
    import concourse.mybir as mybir
    from concourse.tile import TileContext

    dt = mybir.dt
    f32 = dt.float32
    f32r = dt.float32r
    AF = mybir.ActivationFunctionType
    ALU = mybir.AluOpType

    nc = bacc.Bacc(None, target_bir_lowering=False)

    xT_d = nc.dram_tensor("xT", [IN2, MC], f32r, kind="ExternalInput")
    hT_d = nc.dram_tensor("hT", [H, MC], f32r, kind="ExternalInput")
    wiT_d = nc.dram_tensor("wiT", [IN2, G3], f32r, kind="ExternalInput")
    whT_d = nc.dram_tensor("whT", [H, G3], f32r, kind="ExternalInput")
    id_d = nc.dram_tensor("ident", [128, 128], f32r, kind="ExternalInput")
    brz_d = nc.dram_tensor("brz", [2 * H], f32, kind="ExternalInput")
    bin_d = nc.dram_tensor("bin", [H], f32, kind="ExternalInput")
    bhn_d = nc.dram_tensor("bhn", [H], f32, kind="ExternalInput")
    out_d = nc.dram_tensor("houtT", [H, MC], f32, kind="ExternalOutput")

    with TileContext(nc) as tc, ExitStack() as ctx:
        const = ctx.enter_context(tc.tile_pool(name="const", bufs=1))
        xin = ctx.enter_context(tc.tile_pool(name="xin", bufs=3))
        hin = ctx.enter_context(tc.tile_pool(name="hin", bufs=3))
        psum = ctx.enter_context(tc.tile_pool(name="psum", bufs=2, space="PSUM"))
        work = ctx.enter_context(tc.tile_pool(name="work", bufs=3))
        outp = ctx.enter_context(tc.tile_pool(name="outp", bufs=4))

        # --- constants / weights (loaded once) ---
        brz_sb = const.tile([128, 4], f32, tag="brz")
        nc.scalar.dma_start(
            out=brz_sb[:], in_=brz_d[:].rearrange("(g p) -> p g", p=128)
        )
        bin_sb = const.tile([128, 2], f32, tag="bin")
        nc.scalar.dma_start(
            out=bin_sb[:], in_=bin_d[:].rearrange("(g p) -> p g", p=128)
        )
        bhn_sb = const.tile([128, 2], f32, tag="bhn")
        nc.scalar.dma_start(
            out=bhn_sb[:], in_=bhn_d[:].rearrange("(g p) -> p g", p=128)
        )
        wi_sb = const.tile([IN2, G3], f32r, tag="wi")
        nc.sync.dma_start(out=wi_sb[:], in_=wiT_d[:, :])
        # first row-tile data right behind wi so the PE can start early
        x0 = xin.tile([IN2, RT], f32r, tag="x", name="x0")
        nc.sync.dma_start(out=x0[:], in_=xT_d[:, 0:RT])
        h00 = hin.tile([128, RT], f32r, tag="h0", name="h00")
        nc.sync.dma_start(out=h00[:], in_=hT_d[0:128, 0:RT])
        h01 = hin.tile([128, RT], f32r, tag="h1", name="h01")
        nc.sync.dma_start(out=h01[:], in_=hT_d[128:256, 0:RT])
        wh0_sb = const.tile([128, G3], f32r, tag="wh0")
        nc.sync.dma_start(out=wh0_sb[:], in_=whT_d[0:128, :])
        wh1_sb = const.tile([128, G3], f32r, tag="wh1")
        nc.sync.dma_start(out=wh1_sb[:], in_=whT_d[128:256, :])
        id_sb = const.tile([128, 128], f32r, tag="ident")
        nc.sync.dma_start(out=id_sb[:], in_=id_d[:, :])


        for t in range(NRT):
            c0 = t * RT
            if t == 0:
                x_t, h_t = x0, [h00, h01]
            else:
                x_t = xin.tile([IN2, RT], f32r, tag="x")
                nc.sync.dma_start(out=x_t[:], in_=xT_d[:, c0 : c0 + RT])
                h_t = [None, None]
                for j in range(2):
                    h_t[j] = hin.tile([128, RT], f32r, tag=f"h{j}", name=f"h_t{j}")
                    nc.sync.dma_start(
                        out=h_t[j][:], in_=hT_d[j * 128 : (j + 1) * 128, c0 : c0 + RT]
                    )

            def gate_mm(ps, gs):
                nc.tensor.matmul(
                    ps[:], lhsT=wi_sb[:, gs], rhs=x_t[:], start=True, stop=False
                )
                nc.tensor.matmul(
                    ps[:], lhsT=wh0_sb[:, gs], rhs=h_t[0][:], start=False, stop=False
                )
                nc.tensor.matmul(
                    ps[:], lhsT=wh1_sb[:, gs], rhs=h_t[1][:], start=False, stop=True
                )

            def sig(g):
                ps = psum.tile([128, RT], f32, tag="ps", bufs=4, name="ps_rz")
                gate_mm(ps, slice(g * 128, (g + 1) * 128))
                sg = work.tile([128, RT], f32, tag=f"sig{g}", name="sg")
                nc.scalar.activation(
                    out=sg[:], in_=ps[:], func=AF.Sigmoid, bias=brz_sb[:, g : g + 1]
                )
                return sg

            # r gates first: they head the DVE chain.
            r_sig = [sig(0), sig(1)]
            if t == NRT - 1:
                z_sig = [sig(2), sig(3)]
            n_t = [None, None]
            for j in range(2):
                gs = slice(2 * H + j * 128, 2 * H + (j + 1) * 128)
                ps_in = psum.tile([128, RT], f32, tag="psin", bufs=2, name="ps_in")
                nc.tensor.matmul(
                    ps_in[:], lhsT=wi_sb[:, gs], rhs=x_t[:], start=True, stop=False
                )
                ps_hn = psum.tile([128, RT], f32, tag="pshn", bufs=2, name="ps_hn")
                nc.tensor.matmul(
                    ps_hn[:], lhsT=wh0_sb[:, gs], rhs=h_t[0][:], start=True, stop=False
                )
                nc.tensor.matmul(
                    ps_hn[:], lhsT=wh1_sb[:, gs], rhs=h_t[1][:], start=False, stop=True
                )

                t1 = work.tile([128, RT], f32r, tag=f"t1_{j}", name="t1")
                nc.vector.scalar_tensor_tensor(
                    out=t1[:],
                    in0=ps_hn[:],
                    scalar=bhn_sb[:, j : j + 1],
                    in1=r_sig[j][:],
                    op0=ALU.add,
                    op1=ALU.mult,
                )
                nc.tensor.matmul(
                    ps_in[:], lhsT=id_sb[:], rhs=t1[:], start=False, stop=True
                )
                n_t[j] = work.tile([128, RT], f32, tag=f"n_{j}", name="n_t")
                nc.scalar.activation(
                    out=n_t[j][:], in_=ps_in[:], func=AF.Tanh,
                    bias=bin_sb[:, j : j + 1],
                )

            # z gates late: only needed by the final blend.
            if t != NRT - 1:
                z_sig = [sig(2), sig(3)]

            for j in range(2):
                d_t = work.tile([128, RT], f32, tag=f"d_{j}", name="d_t")
                nc.vector.tensor_sub(
                    out=d_t[:], in0=h_t[j][:].bitcast(f32), in1=n_t[j][:]
                )
                e_t = work.tile([128, RT], f32, tag=f"e_{j}", name="e_t")
                nc.vector.tensor_mul(out=e_t[:], in0=z_sig[j][:], in1=d_t[:])
                ho = outp.tile([128, RT], f32, tag=f"ho_{j}", name="ho")
                nc.vector.tensor_add(out=ho[:], in0=n_t[j][:], in1=e_t[:])
                nc.sync.dma_start(
                    out=out_d[j * 128 : (j + 1) * 128, c0 : c0 + RT], in_=ho[:]
                )

    nc.compile()
    return nc


def _get_nc():
    if "nc" not in _compiled:
        _compiled["nc"] = _build_nc()
    return _compiled["nc"]


def _make_in_maps(h, X_obs, i_obs, W_ih, W_hh, b_ih, b_hh):
    f = np.float32
    x = np.asarray(X_obs, f).reshape(M, IN2)
    hs = np.asarray(h, f)[np.asarray(i_obs)]
    xT = np.ascontiguousarray(x.T)
    hT = np.ascontiguousarray(hs.T)
    wiT = np.ascontiguousarray(np.asarray(W_ih, f).T)
    whT = np.ascontiguousarray(np.asarray(W_hh, f).T)
    ident = np.eye(128, dtype=f)
    b_ih = np.asarray(b_ih, f)
    b_hh = np.asarray(b_hh, f)
    brz = (b_ih[: 2 * H] + b_hh[: 2 * H]).astype(f)
    bin_ = np.ascontiguousarray(b_ih[2 * H :])
    bhn = np.ascontiguousarray(b_hh[2 * H :])
    in_maps = []
    for c in range(NCORES):
        cols = slice(c * MC, (c + 1) * MC)
        in_maps.append(
            {
                "xT": np.ascontiguousarray(xT[:, cols]),
                "hT": np.ascontiguousarray(hT[:, cols]),
                "wiT": wiT,
                "whT": whT,
                "ident": ident,
                "brz": brz,
                "bin": bin_,
                "bhn": bhn,
            }
        )
    return in_maps


def run_on_device(h, X_obs, i_obs, W_ih, W_hh, b_ih, b_hh, **run_kwargs):
    """Returns (h_new [M,H] fp32, BassKernelResults)."""
    from concourse.bass_utils import run_bass_kernel_spmd

    in_maps = _make_in_maps(h, X_obs, i_obs, W_ih, W_hh, b_ih, b_hh)
    res = run_bass_kernel_spmd(_get_nc(), in_maps, list(range(NCORES)), **run_kwargs)
    h_new = np.concatenate([r["houtT"].T for r in res.results], axis=0)
    return h_new, res


def kernel(h, X_obs, i_obs, W_ih, W_hh, b_ih, b_hh):
    h = np.asarray(h, np.float32)
    i_obs = np.asarray(i_obs)
    h_new, _ = run_on_device(h, X_obs, i_obs, W_ih, W_hh, b_ih, b_hh)
    out = h.copy()
    out[i_obs] = h_new
    return out
